# revision 30
# baseline (speedup 1.0000x reference)
"""Sliding-window attention kernel for 8 TRN2 NeuronCores.

Sharding: core c owns heads {2c, 2c+1} for BOTH batches (tensor parallel
over the 16 heads).  After attention, an all-to-all redistributes the
per-head outputs so core c owns output rows (batch c//4, t-chunk c%4),
where it applies the full Wo projection.

v2 pipeline (all matmul compute in bf16, fp32 accumulation):
  1. x^T quarters DMA'd in batch-interleaved order; Q^T/K^T projections
     write straight into the attention layout q/k[128=(h2,d), B, T]
     (one ACT copy per PSUM chunk, no rearrange DMAs).
  2. V projected directly in NATURAL layout (lhsT = x^T tile) and copied
     by DVE into the 65-interleaved v_aug (ones column -> softmax
     denominator for free).  No DRAM bounce, no gather DMAs.
  3. RoPE per (proj, b, t-half): one partition-swap DMA + 3 DVE ops.
  4. per (h2, kt, b): banded scores S^T = K'Q'^T (PE) -> exp (ACT) ->
     boundary-triangle masks (DVE) -> E^T in SBUF.
  5. AV^T per (b, qc) -> sender-side softmax normalization: reciprocal of
     the denominator row, broadcast across partitions via a K=1 matmul,
     multiply -> normalized bf16 chunk staged for the all-to-all.
  6. two AllToAll collectives (one per head-half; payload [8, 64, 512]
     bf16, no denominators on the wire).
  7. receiver: one DMA per half into at_sb, Wo in two K=64 passes
     (h2=0 pass overlaps the second collective), f32 accumulation in
     SBUF, DMA out.
"""
import numpy as np
import ml_dtypes

import concourse.bass as bass
import concourse.bacc as bacc
import concourse.mybir as mybir
import concourse.tile as tile

F32 = mybir.dt.float32
BF16 = mybir.dt.bfloat16
AF = mybir.ActivationFunctionType
ALU = mybir.AluOpType

B, T, D = 2, 2048, 1024
H, DH = 16, 64
WIN = T // 4              # 512
N_CORES = 8
HPC = H // N_CORES        # heads per core = 2
TC = T // 4               # output t-chunk per core = 512
KT = T // 128             # k-tiles per (head,batch) = 16
SCALE = 1.0 / np.sqrt(DH)

NKT = 128                 # k-tile rows
MAXW = 1152               # max window width per k-tile


def window(k0):
    """q-window [ws, we) for k-tile starting at k0."""
    return max(k0 - WIN, 0), min(k0 + NKT + WIN, T)


EOFF = []
_off = 0
for _kt in range(KT):
    _ws, _we = max(_kt*128 - WIN, 0), min(_kt*128 + NKT + WIN, T)
    EOFF.append(_off)
    _off += _we - _ws
ETOT = _off


def host_inputs(x, Wq, Wk, Wv, Wo, core):
    """Build the per-core input map (host-side shard + constant tables)."""
    bf = ml_dtypes.bfloat16
    cols = slice(core * HPC * DH, (core + 1) * HPC * DH)
    t = np.arange(T, dtype=np.float64)
    inv = 1.0 / (10000.0 ** (np.arange(0, DH, 2, dtype=np.float64) / DH))
    f = (t[:, None] * inv[None, :]).astype(np.float32)   # [T, 32]
    cos1 = np.cos(f).astype(np.float32)                  # [T, 32]
    sin1 = np.sin(f).astype(np.float32)
    # ^T layout tables [128, T]: row r -> head-dim d = r % 64
    d = np.arange(128) % 64
    cos_t = cos1.T[d % 32]                               # [128, T]
    sin_t = sin1.T[d % 32]
    sgn = np.where(d < 32, -1.0, 1.0).astype(np.float32)[:, None]
    sin_s = sin_t * sgn                                  # signed sin for swap trick
    kr = np.arange(128)[:, None]
    qc = np.arange(128)[None, :]
    tri_l = (kr <= qc).astype(np.float32)                # valid mask, left boundary
    tri_r = (qc <= kr).astype(np.float32)                # valid mask, right boundary
    return {
        "xt": np.ascontiguousarray(x.reshape(B * T, D).T.astype(bf)),
        "wq": np.ascontiguousarray(Wq[:, cols].astype(bf)),
        "wk": np.ascontiguousarray(Wk[:, cols].astype(bf)),
        "wv": np.ascontiguousarray(Wv[:, cols].astype(bf)),
        "wo": np.ascontiguousarray(Wo.astype(bf)),
        "cos_t": cos_t.astype(bf),
        "sin_s": sin_s.astype(bf),
        "tri_l": tri_l.astype(bf),
        "tri_r": tri_r.astype(bf),
    }


def host_assemble(results):
    """Concatenate the 8 per-core [512, 1024] chunks into [B, T, D]."""
    out = np.empty((B, T, D), np.float32)
    for c in range(N_CORES):
        out[c // 4, (c % 4) * TC:(c % 4 + 1) * TC, :] = results[c]["out"]
    return out


def build(nc, replicate=1, debug=False):
    pass  # (debug outputs added below when debug=True)
    x_d = nc.dram_tensor("xt", [D, B * T], BF16, kind="ExternalInput").ap()
    wq_d = nc.dram_tensor("wq", [D, HPC * DH], BF16, kind="ExternalInput").ap()
    wk_d = nc.dram_tensor("wk", [D, HPC * DH], BF16, kind="ExternalInput").ap()
    wv_d = nc.dram_tensor("wv", [D, HPC * DH], BF16, kind="ExternalInput").ap()
    wo_d = nc.dram_tensor("wo", [D, D], BF16, kind="ExternalInput").ap()
    cos_d = nc.dram_tensor("cos_t", [128, T], BF16, kind="ExternalInput").ap()
    sin_d = nc.dram_tensor("sin_s", [128, T], BF16, kind="ExternalInput").ap()
    tl_d = nc.dram_tensor("tri_l", [128, 128], BF16, kind="ExternalInput").ap()
    tr_d = nc.dram_tensor("tri_r", [128, 128], BF16, kind="ExternalInput").ap()
    out_d = nc.dram_tensor("out", [TC, D], F32, kind="ExternalOutput").ap()
    dbg = {}
    if debug:
        for name, shape, dt_ in [
            ("dbg_q", [128, B, T], BF16),
            ("dbg_k", [128, B, T], BF16),
            ("dbg_vaug", [128, B * KT, 130], BF16),
            ("dbg_at", [128, 8, TC], BF16),
        ]:
            dbg[name] = nc.dram_tensor(name, shape, dt_,
                                       kind="ExternalOutput").ap()

    with tile.TileContext(nc) as tc:
        for _ in range(replicate):
            _build_once(nc, tc, x_d, wq_d, wk_d, wv_d, wo_d, cos_d, sin_d,
                        tl_d, tr_d, out_d, dbg)
    nc.compile()
    return nc


def _build_once(nc, tc, x_d, wq_d, wk_d, wv_d, wo_d, cos_d, sin_d, tl_d, tr_d,
                out_d, dbg={}):
    with tc.tile_pool(name="const", bufs=1) as constp, \
         tc.tile_pool(name="xt", bufs=2) as xtp, \
         tc.tile_pool(name="qk", bufs=1) as qkp, \
         tc.tile_pool(name="rope", bufs=1) as ropep, \
         tc.tile_pool(name="ebuf", bufs=2) as ep, \
         tc.tile_pool(name="stg", bufs=2) as stp, \
         tc.tile_pool(name="nrm", bufs=2) as nrmp, \
         tc.tile_pool(name="fin", bufs=1) as finp, \
         tc.tile_pool(name="ps_small", bufs=2, space="PSUM") as pss, \
         tc.tile_pool(name="ps_big", bufs=2, space="PSUM") as psb, \
         tc.tile_pool(name="dram", bufs=1, space="DRAM") as dr:

        # ---------------- constants / weights ----------------
        cos_sb = constp.tile([128, T], BF16, tag="cos")
        sin_sb = constp.tile([128, T], BF16, tag="sin")
        tl_sb = constp.tile([128, 128], BF16, tag="tl")
        tr_sb = constp.tile([128, 128], BF16, tag="tr")
        wq_sb = constp.tile([128, 8, HPC * DH], BF16, tag="wq")
        wk_sb = constp.tile([128, 8, HPC * DH], BF16, tag="wk")
        wv_sb = constp.tile([128, 8, HPC * DH], BF16, tag="wv")
        wo_sb = constp.tile([128, 8, D], BF16, tag="wo")
        ones_sb = constp.tile([1, DH], BF16, tag="ones")
        nc.vector.memset(ones_sb[:], 1.0)
        # wq goes on the SP queue ahead of the first x quarter so PE can
        # start early; wk/wv on the ACT queue (plain per-cb block DMAs)
        for cb in range(8):
            nc.sync.dma_start(wq_sb[:, cb, :], wq_d[cb * 128:(cb + 1) * 128, :])
        for cb in range(8):
            nc.scalar.dma_start(wk_sb[:, cb, :],
                                wk_d[cb * 128:(cb + 1) * 128, :])
            nc.scalar.dma_start(wv_sb[:, cb, :],
                                wv_d[cb * 128:(cb + 1) * 128, :])
        nc.gpsimd.dma_start(cos_sb[:], cos_d[:])
        nc.gpsimd.dma_start(sin_sb[:], sin_d[:])
        nc.gpsimd.dma_start(tl_sb[:], tl_d[:])
        nc.gpsimd.dma_start(tr_sb[:], tr_d[:])

        # q/k in attention layout: partitions = (h2, d), free = (b, t)
        qk_all = {}
        for pn in ("q", "k"):
            qk_all[pn] = qkp.tile([128, B, T], BF16, name=f"{pn}d", tag=f"{pn}d")
        v_aug = qkp.tile([128, B * KT, 130], BF16, tag="vaug")

        a2a_in = [dr.tile([8, DH, TC], BF16, name=f"a2ai{i}", tag=f"a2ai{i}")
                  for i in range(2)]
        a2a_out = [dr.tile([8, DH, TC], BF16, name=f"a2ao{i}", tag=f"a2ao{i}")
                   for i in range(2)]
        at_sb = finp.tile([128, 8, TC], BF16, tag="at")
        # E buffers: exact-width [128, ETOT] per (h2, b) (2 live at a time via
        # the pool), plus a small side buffer for the EARLY h2=1 kt<4 tiles so
        # the exp stream can run during the second half of the projections
        # without 4 full buffers being live.
        e_sb_all = {}
        e_extra = ep.tile([128, B, EOFF[4]], BF16, tag="Ex")

        def make_esb(h2):
            e_sb_all[h2] = {
                b: ep.tile([128, ETOT], BF16, name=f"e{h2}{b}", tag="E")
                for b in range(B)}

        def eslot(h2, b, kt):
            W = window(kt * 128)[1] - window(kt * 128)[0]
            if h2 == 1 and kt < 4:
                return e_extra[:, b, EOFF[kt]:EOFF[kt] + W]
            return e_sb_all[h2][b][:, EOFF[kt]:EOFF[kt] + W]

        def emit_av(h2, b, qc):
            q0 = qc * 512
            kts = [kt for kt in range(KT)
                   if window(kt * 128)[0] < q0 + 512
                   and window(kt * 128)[1] > q0]
            av = pss.tile([65, 512], F32, tag="ps_small",
                          padded_shape=[128, 512])
            for i, kt in enumerate(kts):
                ws, we = window(kt * 128)
                lo = max(q0, ws)
                hi = min(q0 + 512, we)
                nc.tensor.matmul(
                    av[:, lo - q0:hi - q0],
                    v_aug[:, b * KT + kt, 65 * h2:65 * h2 + 65],
                    eslot(h2, b, kt)[:, lo - ws:hi - ws],
                    start=(i == 0), stop=(i == len(kts) - 1))
            # sender-side softmax normalization.  The denominator row sits
            # at partition 64 of the AV psum; engines cannot shift
            # partitions, so copy it out and DMA it down to partition 0,
            # then reciprocal + K=1 broadcast matmul + scale, all at
            # partition 0 with uniform f32 operands.
            dsb = nrmp.tile([65, 512], F32, tag="dsb", bufs=1)
            nc.vector.tensor_copy(dsb[64:65, :], av[64:65, :])
            den0 = nrmp.tile([1, 512], F32, tag="den0")
            nc.gpsimd.dma_start(den0[:], dsb[64:65, :])
            rec = nrmp.tile([1, 512], F32, tag="rec")
            # dsb row is dead after the partition-shift DMA; reuse as scratch
            nc.vector.reciprocal_approx_accurate(
                out=rec[:], in_=den0[:], scratch=dsb[64:65, :])
            rbf = nrmp.tile([1, 512], BF16, tag="rbf")
            nc.vector.tensor_copy(rbf[:], rec[:])
            pb = pss.tile([64, 512], F32, tag="ps_small",
                          padded_shape=[128, 512])
            nc.tensor.matmul(pb[:], ones_sb[:], rbf[:],
                             start=True, stop=True)
            pbf = nrmp.tile([64, 512], F32, tag="pbf")
            nc.vector.tensor_copy(pbf[:], pb[:])
            stage = stp.tile([64, 512], BF16, tag="stg")
            nc.vector.tensor_mul(stage[:], av[0:64, :], pbf[:])
            nc.gpsimd.dma_start(a2a_in[h2][b * 4 + qc, :, :], stage[:])

        def emit_kt(h2, kt, do_av=True):
            k0 = kt * 128
            ws, we = window(k0)
            W = we - ws
            sc = {}
            for b in range(B):
                p0 = 64 * h2
                sc[b] = psb.tile([128, MAXW], F32, name=f"sc{b}",
                                 tag="ps_big")
                lhsT = qk_all["k"][p0:p0 + 64, b, k0:k0 + 128]
                for s0 in range(0, W, 512):
                    s1 = min(s0 + 512, W)
                    nc.tensor.matmul(sc[b][:, s0:s1],
                                     lhsT,
                                     qk_all["q"][p0:p0 + 64, b,
                                                 ws + s0:ws + s1],
                                     start=True, stop=True)
            for b in range(B):
                e = eslot(h2, b, kt)
                nc.scalar.activation(e[:, 0:W], sc[b][:, 0:W],
                                     AF.Exp, scale=SCALE)
                if k0 >= WIN:
                    nc.vector.tensor_mul(e[:, 0:128], e[:, 0:128], tl_sb[:])
                if k0 + 128 + WIN <= T:
                    nc.vector.tensor_mul(
                        e[:, W - 128:W], e[:, W - 128:W], tr_sb[:])
                if do_av:
                    for qc in range(4):
                        if min(4 * qc + 7, KT - 1) == kt:
                            emit_av(h2, b, qc)

        xt_tiles = {}

        def emit_v(qg, ti):
            """V natural-layout projection for t-tile ti (0..7) of quarter."""
            b = qg // 2
            th = qg % 2
            xt = xt_tiles[qg]
            gkt = b * KT + th * 8 + ti
            pv = pss.tile([128, 512], F32, tag="ps_small")
            for cb in range(8):
                nc.tensor.matmul(
                    pv[:, 0:128], xt[:, cb, ti * 128:(ti + 1) * 128],
                    wv_sb[:, cb, :],
                    start=(cb == 0), stop=(cb == 7))
            nc.vector.tensor_copy(v_aug[:, gkt, 0:64], pv[:, 0:64])
            nc.vector.tensor_copy(v_aug[:, gkt, 65:129], pv[:, 64:128])

        def emit_proj(qi, qg, on_dve, with_v):
            b = qg // 2
            th = qg % 2            # t-half within batch
            xt = xtp.tile([128, 8, 1024], BF16, tag="xt")
            xt_tiles[qg] = xt
            for cb in range(8):
                nc.sync.dma_start(
                    xt[:, cb, :], x_d[cb * 128:(cb + 1) * 128,
                                      qg * 1024:(qg + 1) * 1024])
            for half in range(2):
                tt = th * 1024 + half * 512
                for pi, (wsb, pn) in enumerate(
                        ((wq_sb, "q"), (wk_sb, "k"))):
                    if on_dve or (qi * 2 + half + pi) % 2 == 0:
                        pt = pss.tile([128, 512], F32, tag="ps_small")
                    else:
                        pt = psb.tile([128, 512], F32, tag="ps_big",
                                      padded_shape=[128, MAXW])
                    for cb in range(8):
                        nc.tensor.matmul(
                            pt[:], wsb[:, cb, :],
                            xt[:, cb, half * 512:(half + 1) * 512],
                            start=(cb == 0), stop=(cb == 7))
                    # later quarters copy on DVE: ACT is already doing exp
                    if on_dve:
                        nc.vector.tensor_copy(qk_all[pn][:, b, tt:tt + 512],
                                              pt[:])
                    else:
                        nc.scalar.activation(qk_all[pn][:, b, tt:tt + 512],
                                             pt[:], AF.Copy)
                if with_v:
                    for i in range(4):
                        emit_v(qg, half * 4 + i)

        def emit_rope(th):
            c0 = th * 1024
            for pn in ("q", "k"):
                dst = qk_all[pn]
                for bb in range(B):
                    sw = ropep.tile([128, 1024], BF16, tag="sw")
                    # 32-block partition swap via SBUF->SBUF DMA
                    for hh in range(2):
                        p0 = hh * 64
                        nc.gpsimd.dma_start(sw[p0:p0 + 32, :],
                                            dst[p0 + 32:p0 + 64, bb,
                                                c0:c0 + 1024])
                        nc.gpsimd.dma_start(sw[p0 + 32:p0 + 64, :],
                                            dst[p0:p0 + 32, bb,
                                                c0:c0 + 1024])
                    nc.vector.tensor_mul(dst[:, bb, c0:c0 + 1024],
                                         dst[:, bb, c0:c0 + 1024],
                                         cos_sb[:, c0:c0 + 1024])
                    nc.vector.tensor_mul(sw[:], sw[:],
                                         sin_sb[:, c0:c0 + 1024])
                    nc.vector.tensor_add(dst[:, bb, c0:c0 + 1024],
                                         dst[:, bb, c0:c0 + 1024], sw[:])

        # ---------------- emission schedule ----------------
        # quarters in batch-interleaved order so RoPE + early attention can
        # start at the halfway point of the projections.  V for the later
        # quarters is deferred into the attention phase (2 tiles per kt) so
        # it rides in the PE slack under the exp stream instead of delaying
        # the first scores.
        one_view = v_aug[:].rearrange("p k (h e) -> p k h e", e=65)[:, :, :, 64]
        nc.gpsimd.memset(one_view, 1.0)
        emit_proj(0, 0, on_dve=False, with_v=True)
        emit_proj(1, 2, on_dve=False, with_v=True)
        emit_rope(0)
        make_esb(0)
        for kt in range(4):            # windows fully inside t-half 0
            emit_kt(0, kt)
            emit_kt(1, kt)
        emit_proj(2, 1, on_dve=True, with_v=True)
        emit_proj(3, 3, on_dve=True, with_v=True)
        emit_rope(1)
        # wo load deferred to the attention phase (Pool is idle there)
        for cb in range(8):
            nc.gpsimd.dma_start(wo_sb[:, cb, :],
                                wo_d[cb * 128:(cb + 1) * 128, :])
        for kt in range(4, KT):
            emit_kt(0, kt)
        nc.gpsimd.collective_compute(
            "AllToAll", ALU.bypass, replica_groups=[list(range(N_CORES))],
            ins=[a2a_in[0].opt()], outs=[a2a_out[0].opt()])
        make_esb(1)
        for kt in range(4, KT):
            emit_kt(1, kt)
        nc.gpsimd.collective_compute(
            "AllToAll", ALU.bypass, replica_groups=[list(range(N_CORES))],
            ins=[a2a_in[1].opt()], outs=[a2a_out[1].opt()])

        if "dbg_q" in dbg:
            nc.sync.dma_start(dbg["dbg_q"][:], qk_all["q"][:])
            nc.sync.dma_start(dbg["dbg_k"][:], qk_all["k"][:])
            nc.sync.dma_start(dbg["dbg_vaug"][:], v_aug[:])

        # ---------------- receive + Wo ----------------
        # one DMA per half: [8, 64, 512] dram -> at_sb rows 64*h2..
        for h2 in range(2):
            for c in range(8):
                nc.sync.dma_start(at_sb[64 * h2:64 * h2 + 64, c, :],
                                  a2a_out[h2][c, :, :])

        if "dbg_at" in dbg:
            nc.sync.dma_start(dbg["dbg_at"][:], at_sb[:])

        # Wo: single K=128 pass over the full contraction (both head-halves
        # arrive before Wo starts; half the matmul cycles of an h2-split).
        for tt in range(4):
            for mh in range(2):
                po = pss.tile([128, 512], F32, tag="ps_small")
                for c in range(8):
                    nc.tensor.matmul(
                        po[:],
                        at_sb[:, c, tt * 128:(tt + 1) * 128],
                        wo_sb[:, c, mh * 512:(mh + 1) * 512],
                        start=(c == 0), stop=(c == 7))
                ot = stp.tile([128, 512], F32, tag="ot")
                nc.vector.tensor_copy(ot[:], po[:])
                nc.sync.dma_start(
                    out_d[tt * 128:(tt + 1) * 128,
                          mh * 512:(mh + 1) * 512],
                    ot[:])


# ---------------------------------------------------------------------------
# Self-contained entry point: kernel(**inputs) -> full output [2, 2048, 1024]
# ---------------------------------------------------------------------------
_CACHE = {}


def _get_nc():
    if "nc" in _CACHE:
        return _CACHE["nc"]
    import concourse.bacc as _bacc
    nc = _bacc.Bacc("TRN2", target_bir_lowering=False, debug=False,
                    num_devices=N_CORES)
    build(nc)
    _CACHE["nc"] = nc
    return nc


def kernel(x, Wq, Wk, Wv, Wo):
    from concourse.bass_utils import run_bass_kernel_spmd
    x, Wq, Wk, Wv, Wo = (np.asarray(a, np.float32) for a in (x, Wq, Wk, Wv, Wo))
    nc = _get_nc()
    in_maps = [host_inputs(x, Wq, Wk, Wv, Wo, c) for c in range(N_CORES)]
    res = run_bass_kernel_spmd(nc, in_maps, core_ids=list(range(N_CORES)))
    return host_assemble(res.results)
